# revision 1
# baseline (speedup 1.0000x reference)
"""Trainium2 Bass kernel for 4-bit-quantized Linear: y = x @ dequant(Wq4).T + bias.

Sharding: tensor-parallel over out_features (11008 rows -> 8 cores x 1408,
last core zero-padded), x replicated (fed pre-transposed fp16), outputs
concatenated on host.

Per-core device kernel:
  - dequant int4 (packed 2-nibbles-per-int32) -> fp16 weights, scaled by
    per-block norm:  W = (2*q - 15) * (norm/15)
  - PE-transpose dequantized [o,k] tiles into K-major [k,o] layout
  - fp16 matmul (PSUM fp32 accumulation over K=4096) + bias add
Output columns are processed in 3 chunks (512/512/384) so chunk c+1's
dequant overlaps chunk c's matmuls.
"""
import os
import numpy as np

import concourse.bass as bass
import concourse.bacc as bacc
import concourse.mybir as mybir
import concourse.tile as tile
from concourse.bass_utils import run_bass_kernel_spmd

F16, F32, I32 = mybir.dt.float16, mybir.dt.float32, mybir.dt.int32

# Problem constants (hardcoded per contract)
TOKENS, IN, OUT = 4096, 4096, 11008
GROUP, BLOCKS, HALF = 16, 256, 8
N_CORES = 8
O_C = 1408                      # padded per-core out rows (11 tiles of 128)
KT = IN // 128                  # 32 k-slabs
TC = 256                        # t super-chunk
O_CHUNKS = [(0, 512), (512, 512), (1024, 384)]   # (offset, width); 128-tile aligned


def build_bass(tokens=TOKENS, in_=IN, o_c=O_C, tc_sz=TC, o_chunks=None, reps=1):
    """Build the per-core Bass program (parameterized for small-scale sim tests)."""
    kt = in_ // 128
    blocks = in_ // GROUP
    if o_chunks is None:
        o_chunks = O_CHUNKS
    max_w = max(w for _, w in o_chunks)
    n_tc = tokens // tc_sz
    tl_per_tc = tc_sz // 128

    nc = bacc.Bacc("TRN2", target_bir_lowering=False, debug=False)

    xt_d = nc.dram_tensor("xt", [tokens // tc_sz, 128, (in_ // 128) * tc_sz], F16, kind="ExternalInput")
    wq_d = nc.dram_tensor("wq", [o_c, blocks * HALF], I32, kind="ExternalInput")
    wn_d = nc.dram_tensor("wn", [o_c, blocks], F16, kind="ExternalInput")
    br_d = nc.dram_tensor("bias_rep", [128, o_c], F32, kind="ExternalInput")
    id_d = nc.dram_tensor("ident", [128, 128], F16, kind="ExternalInput")
    y_d = nc.dram_tensor("y", [tokens, o_c], F32, kind="ExternalOutput")

    with tile.TileContext(nc) as tc:
        with (
            tc.tile_pool(name="const", bufs=1) as cst,
            tc.tile_pool(name="dq", bufs=1) as dq,
            tc.tile_pool(name="dqv", bufs=1) as dqv,
            tc.tile_pool(name="wt", bufs=1) as wtp,
            tc.tile_pool(name="xp", bufs=2) as xp,
            tc.tile_pool(name="yp", bufs=2) as yp,
            tc.tile_pool(name="pst", bufs=2, space=bass.MemorySpace.PSUM) as pst,
            tc.tile_pool(name="psm", bufs=2, space=bass.MemorySpace.PSUM) as psm,
        ):
            ident = cst.tile([128, 128], F16, tag="ident")
            nc.gpsimd.dma_start(ident[:], id_d[:])
            bias_sb = cst.tile([128, o_c], F32, tag="bias")
            nc.gpsimd.dma_start(bias_sb[:], br_d[:])

            wts = []
            for oc_i, (o_off, o_w) in enumerate(o_chunks):
                n_ot = o_w // 128
                # ---------------- dequant this chunk's o-tiles ----------------
                wtc = wtp.tile([128, kt, max_w], F16, tag=f"wtc{oc_i}")
                wts.append(wtc)
                for oti in range(n_ot):
                    ot = o_off // 128 + oti
                    v = dqv.tile([128, blocks, HALF], I32, tag="v")
                    nc.gpsimd.dma_start(
                        v[:], wq_d[ot * 128:(ot + 1) * 128].rearrange(
                            "o (b h) -> o b h", h=HALF))
                    nrm = dqv.tile([128, blocks], F16, tag="nrm")
                    nc.gpsimd.dma_start(nrm[:], wn_d[ot * 128:(ot + 1) * 128])
                    s = dq.tile([128, blocks], F32, tag="s")
                    nc.vector.tensor_scalar_mul(s[:], nrm[:], 1.0 / 15.0)

                    a = dq.tile([128, blocks, HALF], I32, tag="a")
                    zq = dq.tile([128, blocks, GROUP], F16, tag="zq")
                    # lo nibble -> even g, hi nibble -> odd g; z = 2*q - 15
                    nc.vector.tensor_scalar(
                        a[:], v[:], 15, None, mybir.AluOpType.bitwise_and)
                    nc.scalar.activation(
                        zq[:, :, 0::2], a[:],
                        mybir.ActivationFunctionType.Copy, bias=-15.0, scale=2.0)
                    nc.vector.tensor_scalar(
                        a[:], v[:], 4, None, mybir.AluOpType.logical_shift_right)
                    nc.scalar.activation(
                        zq[:, :, 1::2], a[:],
                        mybir.ActivationFunctionType.Copy, bias=-15.0, scale=2.0)
                    # W = z * (norm/15), broadcast norm over the group dim
                    s_b = bass.AP(s[:].tensor, s[:].offset, s[:].ap + [[0, GROUP]])
                    nc.vector.tensor_tensor(
                        zq[:], zq[:], s_b, mybir.AluOpType.mult)

                    # transpose [o,k] -> [k,o] via PE, up to 4 tiles per PSUM bank
                    tb = min(4, kt)
                    for c4 in range((kt + tb - 1) // tb):
                        pt = pst.tile([128, tb, 128], F16, tag="pt")
                        ks = [c4 * tb + j for j in range(tb) if c4 * tb + j < kt]
                        for j, k in enumerate(ks):
                            nc.tensor.transpose(
                                pt[:, j, :], zq[:, k * 8:(k + 1) * 8, :], ident[:])
                        # one strided copy drains the whole bank: dest strided over k
                        dst = bass.AP(
                            wtc[:].tensor, wtc[:].offset
                            + ks[0] * max_w + oti * 128,
                            [wtc[:].ap[0], [max_w, len(ks)], [1, 128]])
                        nc.scalar.copy(dst, pt[:, :len(ks), :])

            # ---------------- matmul: single pass over x ----------------
            for rep in range(reps):
                for tci in range(n_tc):
                    xtt = xp.tile([128, kt, tc_sz], F16, tag="xtt")
                    nc.gpsimd.dma_start(
                        xtt[:], xt_d[tci].rearrange("p (s t) -> p s t", s=kt))
                    y_sb = yp.tile([128, tl_per_tc, o_c], F32, tag="y")
                    for tl in range(tl_per_tc):
                        pss = []
                        for i in range(len(o_chunks)):
                            ps_t = psm.tile([128, max_w], F32, tag=f"ps{i}")
                            pss.append(ps_t)
                        for k in range(kt):
                            for ci, (o_off, o_w) in enumerate(o_chunks):
                                nc.tensor.matmul(
                                    pss[ci][:, :o_w],
                                    xtt[:, k, tl * 128:(tl + 1) * 128],
                                    wts[ci][:, k, :o_w],
                                    start=(k == 0), stop=(k == kt - 1))
                        for ci, (o_off, o_w) in enumerate(o_chunks):
                            nc.vector.tensor_tensor(
                                y_sb[:, tl, o_off:o_off + o_w], pss[ci][:, :o_w],
                                bias_sb[:, o_off:o_off + o_w], mybir.AluOpType.add)
                    nc.gpsimd.dma_start(
                        y_d[tci * tc_sz:(tci + 1) * tc_sz, :]
                        .rearrange("(l p) o -> p l o", p=128),
                        y_sb[:])
    nc.compile()
    return nc


def _prep_host_inputs(x, weight_q4, weight_norm, bias):
    """Host-side shard + layout prep. Returns in_maps for 8 cores."""
    n_tc = TOKENS // TC
    xt = (x.T.astype(np.float16).reshape(KT, 128, n_tc, TC)
          .transpose(2, 1, 0, 3).reshape(n_tc, 128, KT * TC))
    xt = np.ascontiguousarray(xt)
    o_pad = N_CORES * O_C
    wq = np.zeros((o_pad, BLOCKS * HALF), np.int32)
    wq[:OUT] = weight_q4.reshape(OUT, BLOCKS * HALF)
    wn = np.zeros((o_pad, BLOCKS), np.float16)
    wn[:OUT] = weight_norm.reshape(OUT, BLOCKS).astype(np.float16)
    bs = np.zeros((o_pad,), np.float32)
    bs[:OUT] = bias
    ident = np.eye(128, dtype=np.float16)

    in_maps = []
    for c in range(N_CORES):
        sl = slice(c * O_C, (c + 1) * O_C)
        in_maps.append({
            "xt": xt,
            "wq": np.ascontiguousarray(wq[sl]),
            "wn": np.ascontiguousarray(wn[sl]),
            "bias_rep": np.ascontiguousarray(
                np.broadcast_to(bs[sl][None, :], (128, O_C))),
            "ident": ident,
        })
    return in_maps


_CACHE = {}


def _run(in_maps):
    if "nc" not in _CACHE:
        _CACHE["nc"] = build_bass()
    nc = _CACHE["nc"]
    res = run_bass_kernel_spmd(nc, in_maps, list(range(N_CORES)))
    return res


def kernel(x, weight_q4, weight_norm, bias):
    in_maps = _prep_host_inputs(
        np.asarray(x), np.asarray(weight_q4),
        np.asarray(weight_norm), np.asarray(bias))
    res = _run(in_maps)
    outs = [res.results[c]["y"] for c in range(N_CORES)]
    y = np.concatenate(outs, axis=1)[:, :OUT]
    return np.ascontiguousarray(y.astype(np.float32))



# revision 3
# speedup vs baseline: 1.1383x; 1.1383x over previous
"""Trainium2 Bass kernel for 4-bit-quantized Linear: y = x @ dequant(Wq4).T + bias.

Sharding: tensor-parallel over out_features (11008 rows -> 8 cores x 1376,
exact split), x replicated (fed pre-transposed fp16), outputs concatenated
on host.

Per-core device kernel (v2):
  - Host pre-unpacks the int4 nibbles to a transposed uint8 layout
    wqT[k, o] (pure layout transform; values stay 4-bit codes), plus a
    norm replication s[k, o] = norm[o, k//16] (fp16).
  - Device dequant is k-major, PE-free: ACT z = (2q-15)/15, DVE W = z*s,
    written straight into the fp16 weight slabs the matmul streams from.
  - Matmul is chunk-outer (o-chunks 512/512/352): pass over all tokens for
    one chunk while the next chunk dequantizes in the background; x is
    re-read per pass (DMA has headroom; PE is the bottleneck).
  - fp16 matmul (PSUM fp32 accumulation over K=4096) + bias add.
"""
import numpy as np

import concourse.bass as bass
import concourse.bacc as bacc
import concourse.mybir as mybir
import concourse.tile as tile
from concourse.bass_utils import run_bass_kernel_spmd

F16, F32, U8 = mybir.dt.float16, mybir.dt.float32, mybir.dt.uint8

# Problem constants (hardcoded per contract)
TOKENS, IN, OUT = 4096, 4096, 11008
GROUP, BLOCKS, HALF = 16, 256, 8
N_CORES = 8
O_C = OUT // N_CORES            # 1376 per-core out rows, exact
KT = IN // 128                  # 32 k-slabs
TC = 256                        # tokens per x-DMA super-chunk
O_CHUNKS = [(0, 512), (512, 512), (1024, 352)]   # (offset, width)
MAXW = 512


def build_bass(tokens=TOKENS, in_=IN, o_c=O_C, tc_sz=TC, o_chunks=None):
    kt = in_ // 128
    if o_chunks is None:
        o_chunks = O_CHUNKS
    n_tc = tokens // tc_sz
    tl_per_tc = tc_sz // 128
    maxw = max(w for _, w in o_chunks)

    nc = bacc.Bacc("TRN2", target_bir_lowering=False, debug=False)

    xt_d = nc.dram_tensor("xt", [n_tc, 128, kt * tc_sz], F16, kind="ExternalInput")
    wq_d = nc.dram_tensor("wq", [kt, 128, o_c], U8, kind="ExternalInput")
    sc_d = nc.dram_tensor("sc", [kt, 128, o_c], F16, kind="ExternalInput")
    br_d = nc.dram_tensor("bias_rep", [128, o_c], F32, kind="ExternalInput")
    y_d = nc.dram_tensor("y", [tokens, o_c], F32, kind="ExternalOutput")

    with tile.TileContext(nc) as tc:
        with (
            tc.tile_pool(name="const", bufs=1) as cst,
            tc.tile_pool(name="wp", bufs=2) as wp,
            tc.tile_pool(name="q8", bufs=4) as q8p,
            tc.tile_pool(name="sc", bufs=4) as scp,
            tc.tile_pool(name="zz", bufs=4) as zzp,
            tc.tile_pool(name="xp", bufs=3) as xp,
            tc.tile_pool(name="yp", bufs=4) as yp,
            tc.tile_pool(name="psm", bufs=4, space=bass.MemorySpace.PSUM) as psm,
        ):
            bias_sb = cst.tile([128, o_c], F32, tag="bias")
            nc.gpsimd.dma_start(bias_sb[:], br_d[:])

            wts = {}

            def dq_slab(ci, k, first_chunk):
                """Dequantize one k-slab of chunk ci into wts[ci]."""
                o_off, o_w = o_chunks[ci]
                q = q8p.tile([128, maxw], U8, tag="q")
                nc.scalar.dma_start(
                    q[:, :o_w], wq_d[k, :, o_off:o_off + o_w])
                s = scp.tile([128, maxw], F16, tag="s")
                # chunk0's scales ride the (otherwise idle) sync queue so
                # both HW queues fill the critical head; later chunks use
                # scalar to keep sync free for x.
                eng = nc.sync if first_chunk else nc.scalar
                eng.dma_start(s[:, :o_w], sc_d[k, :, o_off:o_off + o_w])
                z = zzp.tile([128, maxw], F16, tag="z")
                # z = (2q - 15)/15 = q*(2/15) - 1
                nc.scalar.activation(
                    z[:, :o_w], q[:, :o_w],
                    mybir.ActivationFunctionType.Copy, bias=-1.0, scale=2.0 / 15.0)
                nc.vector.tensor_tensor(
                    wts[ci][:, k, :o_w], z[:, :o_w], s[:, :o_w],
                    mybir.AluOpType.mult)

            # chunk 0 dequant up front (matmuls stagger in behind it)
            wts[0] = wp.tile([128, kt, maxw], F16, tag="W", name="W0")
            for k in range(kt):
                dq_slab(0, k, True)

            for ci, (o_off, o_w) in enumerate(o_chunks):
                if ci + 1 < len(o_chunks):
                    wts[ci + 1] = wp.tile([128, kt, maxw], F16, tag="W", name=f"W{ci+1}")
                for tci in range(n_tc):
                    xtt = xp.tile([128, kt, tc_sz], F16, tag="xtt")
                    nc.sync.dma_start(
                        xtt[:], xt_d[tci].rearrange("p (s t) -> p s t", s=kt))
                    for tl in range(tl_per_tc):
                        ps = psm.tile([128, maxw], F32, tag="ps")
                        for k in range(kt):
                            nc.tensor.matmul(
                                ps[:, :o_w],
                                xtt[:, k, tl * 128:(tl + 1) * 128],
                                wts[ci][:, k, :o_w],
                                start=(k == 0), stop=(k == kt - 1))
                        y_sb = yp.tile([128, maxw], F32, tag="y")
                        nc.vector.tensor_tensor(
                            y_sb[:, :o_w], ps[:, :o_w],
                            bias_sb[:, o_off:o_off + o_w], mybir.AluOpType.add)
                        nc.gpsimd.dma_start(
                            y_d[tci * tc_sz + tl * 128:
                                tci * tc_sz + (tl + 1) * 128,
                                o_off:o_off + o_w],
                            y_sb[:, :o_w])
                    # trickle next chunk's dequant under this pass
                    if ci + 1 < len(o_chunks):
                        for k in range(2 * tci, 2 * tci + 2):
                            if k < kt:
                                dq_slab(ci + 1, k, False)
    nc.compile()
    return nc


def _prep_host_inputs(x, weight_q4, weight_norm, bias):
    """Host-side shard + layout prep. Returns in_maps for 8 cores."""
    n_tc = TOKENS // TC
    xt = (x.T.astype(np.float16).reshape(KT, 128, n_tc, TC)
          .transpose(2, 1, 0, 3).reshape(n_tc, 128, KT * TC))
    xt = np.ascontiguousarray(xt)

    # nibble-unpack + transpose: wqT[k, o] = 4-bit code of W[o, k]
    b = weight_q4.reshape(OUT, BLOCKS * HALF).astype(np.uint8)
    q = np.empty((OUT, IN), np.uint8)
    q[:, 0::2] = b & 15
    q[:, 1::2] = b >> 4
    qT = np.ascontiguousarray(q.T).reshape(KT, 128, OUT)

    # scale replication: s[k, o] = norm[o, k//16]
    sT = np.repeat(
        weight_norm.reshape(OUT, BLOCKS).T.astype(np.float16),
        GROUP, axis=0).reshape(KT, 128, OUT)

    bias = bias.astype(np.float32)

    in_maps = []
    for c in range(N_CORES):
        sl = slice(c * O_C, (c + 1) * O_C)
        in_maps.append({
            "xt": xt,
            "wq": np.ascontiguousarray(qT[:, :, sl]),
            "sc": np.ascontiguousarray(sT[:, :, sl]),
            "bias_rep": np.ascontiguousarray(
                np.broadcast_to(bias[sl][None, :], (128, O_C))),
        })
    return in_maps


_CACHE = {}


def _run(in_maps):
    if "nc" not in _CACHE:
        _CACHE["nc"] = build_bass()
    nc = _CACHE["nc"]
    res = run_bass_kernel_spmd(nc, in_maps, list(range(N_CORES)))
    return res


def kernel(x, weight_q4, weight_norm, bias):
    in_maps = _prep_host_inputs(
        np.asarray(x), np.asarray(weight_q4),
        np.asarray(weight_norm), np.asarray(bias))
    res = _run(in_maps)
    outs = [res.results[c]["y"] for c in range(N_CORES)]
    y = np.concatenate(outs, axis=1)
    return np.ascontiguousarray(y.astype(np.float32))


# revision 4
# speedup vs baseline: 1.1661x; 1.0244x over previous
"""Trainium2 Bass kernel for 4-bit-quantized Linear: y = x @ dequant(Wq4).T + bias.

Sharding: tensor-parallel over out_features (11008 rows -> 8 cores x 1376,
exact split), x replicated (fed pre-transposed fp16), outputs concatenated
on host.

Per-core device kernel (v2):
  - Host pre-unpacks the int4 nibbles to a transposed uint8 layout
    wqT[k, o] (pure layout transform; values stay 4-bit codes), plus a
    norm replication s[k, o] = norm[o, k//16] (fp16).
  - Device dequant is k-major, PE-free: ACT z = (2q-15)/15, DVE W = z*s,
    written straight into the fp16 weight slabs the matmul streams from.
  - Matmul is chunk-outer (o-chunks 512/512/352): pass over all tokens for
    one chunk while the next chunk dequantizes in the background; x is
    re-read per pass (DMA has headroom; PE is the bottleneck).
  - fp16 matmul (PSUM fp32 accumulation over K=4096) + bias add.
"""
import numpy as np

import concourse.bass as bass
import concourse.bacc as bacc
import concourse.mybir as mybir
import concourse.tile as tile
from concourse.bass_utils import run_bass_kernel_spmd

F16, F32, U8 = mybir.dt.float16, mybir.dt.float32, mybir.dt.uint8

# Problem constants (hardcoded per contract)
TOKENS, IN, OUT = 4096, 4096, 11008
GROUP, BLOCKS, HALF = 16, 256, 8
N_CORES = 8
O_C = OUT // N_CORES            # 1376 per-core out rows, exact
KT = IN // 128                  # 32 k-slabs
TC = 256                        # tokens per x-DMA super-chunk
O_CHUNKS = [(0, 512), (512, 512), (1024, 352)]   # (offset, width)
MAXW = 512


def build_bass(tokens=TOKENS, in_=IN, o_c=O_C, tc_sz=TC, o_chunks=None):
    kt = in_ // 128
    if o_chunks is None:
        o_chunks = O_CHUNKS
    n_tc = tokens // tc_sz
    tl_per_tc = tc_sz // 128
    maxw = max(w for _, w in o_chunks)

    nc = bacc.Bacc("TRN2", target_bir_lowering=False, debug=False)

    xt_d = nc.dram_tensor("xt", [n_tc, 128, kt * tc_sz], F16, kind="ExternalInput")
    wq_d = nc.dram_tensor("wq", [kt, 128, o_c], U8, kind="ExternalInput")
    sc_d = nc.dram_tensor("sc", [kt, 128, o_c], F16, kind="ExternalInput")
    br_d = nc.dram_tensor("bias_rep", [128, o_c], F32, kind="ExternalInput")
    y_d = nc.dram_tensor("y", [tokens, o_c], F32, kind="ExternalOutput")

    with tile.TileContext(nc) as tc:
        with (
            tc.tile_pool(name="const", bufs=1) as cst,
            tc.tile_pool(name="wp", bufs=2) as wp,
            tc.tile_pool(name="q8", bufs=4) as q8p,
            tc.tile_pool(name="sc", bufs=4) as scp,
            tc.tile_pool(name="zz", bufs=4) as zzp,
            tc.tile_pool(name="xp", bufs=3) as xp,
            tc.tile_pool(name="yp", bufs=4) as yp,
            tc.tile_pool(name="psm", bufs=4, space=bass.MemorySpace.PSUM) as psm,
        ):
            wts = {}

            def dq_slab(ci, k, first_chunk):
                """Dequantize one k-slab of chunk ci into wts[ci]."""
                o_off, o_w = o_chunks[ci]
                # head-critical chunk0 alternates its DMAs across the two
                # non-x queues; later chunks ride scalar only, keeping sync
                # mostly free for x.
                eng = (nc.gpsimd if (first_chunk and k % 2 == 0)
                       else nc.scalar)
                q = q8p.tile([128, maxw], U8, tag="q")
                eng.dma_start(q[:, :o_w], wq_d[k, :, o_off:o_off + o_w])
                s = scp.tile([128, maxw], F16, tag="s")
                eng.dma_start(s[:, :o_w], sc_d[k, :, o_off:o_off + o_w])
                z = zzp.tile([128, maxw], F16, tag="z")
                # z = (2q - 15)/15 = q*(2/15) - 1
                nc.scalar.activation(
                    z[:, :o_w], q[:, :o_w],
                    mybir.ActivationFunctionType.Copy, bias=-1.0, scale=2.0 / 15.0)
                nc.vector.tensor_tensor(
                    wts[ci][:, k, :o_w], z[:, :o_w], s[:, :o_w],
                    mybir.AluOpType.mult)

            def x_dma(xtt, tci):
                eng = nc.sync if tci % 2 == 0 else nc.scalar
                eng.dma_start(
                    xtt[:], xt_d[tci].rearrange("p (s t) -> p s t", s=kt))

            # first x tile leads the sync queue; chunk0 dequant staggers in
            xtt0 = xp.tile([128, kt, tc_sz], F16, tag="xtt", name="xtt0")
            x_dma(xtt0, 0)
            bias_sb = cst.tile([128, o_c], F32, tag="bias")
            nc.sync.dma_start(bias_sb[:], br_d[:])
            wts[0] = wp.tile([128, kt, maxw], F16, tag="W", name="W0")
            for k in range(kt):
                dq_slab(0, k, True)

            for ci, (o_off, o_w) in enumerate(o_chunks):
                if ci + 1 < len(o_chunks):
                    wts[ci + 1] = wp.tile([128, kt, maxw], F16, tag="W", name=f"W{ci+1}")
                for tci in range(n_tc):
                    if ci == 0 and tci == 0:
                        xtt = xtt0
                    else:
                        xtt = xp.tile([128, kt, tc_sz], F16, tag="xtt")
                        x_dma(xtt, tci)
                    for tl in range(tl_per_tc):
                        ps = psm.tile([128, maxw], F32, tag="ps")
                        for k in range(kt):
                            nc.tensor.matmul(
                                ps[:, :o_w],
                                xtt[:, k, tl * 128:(tl + 1) * 128],
                                wts[ci][:, k, :o_w],
                                start=(k == 0), stop=(k == kt - 1))
                        y_sb = yp.tile([128, maxw], F32, tag="y")
                        nc.vector.tensor_tensor(
                            y_sb[:, :o_w], ps[:, :o_w],
                            bias_sb[:, o_off:o_off + o_w], mybir.AluOpType.add)
                        yeng = nc.sync if tl % 2 == 0 else nc.gpsimd
                        yeng.dma_start(
                            y_d[tci * tc_sz + tl * 128:
                                tci * tc_sz + (tl + 1) * 128,
                                o_off:o_off + o_w],
                            y_sb[:, :o_w])
                    # trickle next chunk's dequant under this pass
                    if ci + 1 < len(o_chunks):
                        for k in range(2 * tci, 2 * tci + 2):
                            if k < kt:
                                dq_slab(ci + 1, k, False)
    nc.compile()
    return nc


def _prep_host_inputs(x, weight_q4, weight_norm, bias):
    """Host-side shard + layout prep. Returns in_maps for 8 cores."""
    n_tc = TOKENS // TC
    xt = (x.T.astype(np.float16).reshape(KT, 128, n_tc, TC)
          .transpose(2, 1, 0, 3).reshape(n_tc, 128, KT * TC))
    xt = np.ascontiguousarray(xt)

    # nibble-unpack + transpose: wqT[k, o] = 4-bit code of W[o, k]
    b = weight_q4.reshape(OUT, BLOCKS * HALF).astype(np.uint8)
    q = np.empty((OUT, IN), np.uint8)
    q[:, 0::2] = b & 15
    q[:, 1::2] = b >> 4
    qT = np.ascontiguousarray(q.T).reshape(KT, 128, OUT)

    # scale replication: s[k, o] = norm[o, k//16]
    sT = np.repeat(
        weight_norm.reshape(OUT, BLOCKS).T.astype(np.float16),
        GROUP, axis=0).reshape(KT, 128, OUT)

    bias = bias.astype(np.float32)

    in_maps = []
    for c in range(N_CORES):
        sl = slice(c * O_C, (c + 1) * O_C)
        in_maps.append({
            "xt": xt,
            "wq": np.ascontiguousarray(qT[:, :, sl]),
            "sc": np.ascontiguousarray(sT[:, :, sl]),
            "bias_rep": np.ascontiguousarray(
                np.broadcast_to(bias[sl][None, :], (128, O_C))),
        })
    return in_maps


_CACHE = {}


def _run(in_maps):
    if "nc" not in _CACHE:
        _CACHE["nc"] = build_bass()
    nc = _CACHE["nc"]
    res = run_bass_kernel_spmd(nc, in_maps, list(range(N_CORES)))
    return res


def kernel(x, weight_q4, weight_norm, bias):
    in_maps = _prep_host_inputs(
        np.asarray(x), np.asarray(weight_q4),
        np.asarray(weight_norm), np.asarray(bias))
    res = _run(in_maps)
    outs = [res.results[c]["y"] for c in range(N_CORES)]
    y = np.concatenate(outs, axis=1)
    return np.ascontiguousarray(y.astype(np.float32))
